# revision 8
# baseline (speedup 1.0000x reference)
"""Trainium2 Bass kernel for CasAttention2D — v3.

Math (reference):
    kh  = k @ Wk;  v = kh @ Wv;  qh = q @ Wq
    ph  = relu(pos @ P1 + pb1) @ P2 + pb2
    s   = kh - qh[:,:,None,:] + ph
    a   = relu(s @ A1 + ab1) @ A2 + ab2
    a   = where(mask==0, -1e9, a); attn = softmax(a, axis=K)
    out = ((v + ph) * attn).sum(K) @ Wo + bo

Device-side reformulation (feature-major; per k-row r = (token, k)):
    host: r1  = relu(pos@P1 + pb1)                         [R, H]
          qab = -q@(Wq A1) + (ab1 + pb2@A1), repeated K    [R, H]
          rq  = [r1; qab]^T  (bf16)                        [2H, R]
          mc  = (1 - mask)   (bf16)                        [R]
    dev:  s1  = (Wk A1)^T kt + [P2 A1; I]^T rq             [H, cols]
          a1  = relu(s1)   (4 chunks stacked at PE col-tiles 0/32/64/96,
                            one Act op per stack)
          a2  = [A2; -1e9]^T [a1; mc]                      [D, cols]
          num = exp(a2)            (masked rows underflow to exactly 0)
          vph = (Wk Wv)^T kt + P2^T rq[:H]
          den = treeadd_K(num)          [4 TT-adds on GPSIMD,
                                         2-chunk [D,1024] batches]
          ynum= treeadd_K(vph * num)    [4 bf16 TT-adds on DVE (2x mode),
                                         2-chunk batches]
          xsup= ynum * (1/den);  out = Wo^T xsup + (bo + pb2@Wo)
          (group tails deferred 3 chunks so the in-order PE queue
           never parks on the DVE/GPSIMD tail chain)
"""

import numpy as np
from contextlib import ExitStack

import sys

for _p in ("/root/.axon_site/_ro/trn_rl_repo", "/root/.axon_site/_ro/pypackages",
           "/opt/trn_rl_repo", "/opt/pypackages"):
    if _p not in sys.path:
        sys.path.append(_p)

import concourse.bass as bass
import concourse.tile as tile
from concourse import mybir
from concourse.bass_utils import run_bass_kernel_spmd

# problem dims (hardcoded per contract)
B, N, K, D = 4, 4096, 16, 128
H = D // 8
NCORES = 8
T_TOTAL = B * N                 # 16384 tokens
T_CORE = T_TOTAL // NCORES      # 2048 tokens per core
R_CORE = T_CORE * K             # 32768 k-rows per core
CHUNK = 512                     # k-rows per chunk (32 tokens)
TOK_CHUNK = CHUNK // K          # 32 tokens per chunk
NCHUNK = R_CORE // CHUNK        # 64
STACK = 4                       # chunks per relu stack (slots at 0/32/64/96)
SLOT = 32                       # partition stride between stack slots
NSTACK = NCHUNK // STACK        # 16
GRP = 8                         # chunks per output group (256 tokens)
TOK_GRP = GRP * TOK_CHUNK       # 256
DMAB = 4                        # chunks per input DMA batch

F32 = mybir.dt.float32
F32R = mybir.dt.float32r
BF16 = mybir.dt.bfloat16
AF = mybir.ActivationFunctionType
ALU = mybir.AluOpType

def _legalize_waits(nc):
    """This walrus build encodes at most ONE sync-wait per instruction.
    Split multi-wait instructions into single-wait same-engine NoOps."""
    cnt = 0
    for fn in nc.m.functions:
        for blk in fn.blocks:
            bb = blk.bb if hasattr(blk, "bb") else blk
            insts = bb.instructions
            new_list = []
            for inst in insts:
                si = inst.sync_info
                waits = list(si.on_wait) if (si and si.on_wait) else []
                if len(waits) > 1:
                    for w in waits[:-1]:
                        cnt += 1
                        nop = mybir.InstNoOp(
                            name=f"WSPLIT-{cnt}-{inst.name}",
                            sync_info=mybir.SyncInfo(on_wait=[w], on_update=[]),
                        )
                        nop.engine = inst.engine
                        new_list.append(nop)
                    si.on_wait = [waits[-1]]
                new_list.append(inst)
            del insts[:]
            for x in new_list:
                insts.append(x)
    return cnt


def _build_program(uadd_chunks=(), reps=1):
    nc = bass.Bass()

    kf = nc.dram_tensor("kf", [D, R_CORE], BF16, kind="ExternalInput")
    rqf = nc.dram_tensor("rqf", [2 * H, R_CORE], BF16, kind="ExternalInput")
    mrows = nc.dram_tensor("mrows", [NCHUNK, CHUNK], BF16, kind="ExternalInput")
    uadd = nc.dram_tensor("uadd", [1, R_CORE], BF16, kind="ExternalInput")

    w_kv = nc.dram_tensor("w_kv", [D, D], BF16, kind="ExternalInput")
    w_ka = nc.dram_tensor("w_ka", [D, H], BF16, kind="ExternalInput")
    w_p2ai = nc.dram_tensor("w_p2ai", [2 * H, H], BF16, kind="ExternalInput")
    w_p2 = nc.dram_tensor("w_p2", [H, D], BF16, kind="ExternalInput")
    # A2m replicated at partitions 0/32/64/96 (PE tile positions)
    w_a2m = nc.dram_tensor("w_a2m", [D, D], BF16, kind="ExternalInput")
    w_o = nc.dram_tensor("w_o", [D, D], F32R, kind="ExternalInput")
    b_o = nc.dram_tensor("b_o", [D, 1], F32, kind="ExternalInput")

    out_f = nc.dram_tensor("out_f", [D, T_CORE], F32, kind="ExternalOutput")

    with ExitStack() as ctx:
        tc = ctx.enter_context(tile.TileContext(nc))
        consts = ctx.enter_context(tc.tile_pool(name="consts", bufs=1))
        kpool = ctx.enter_context(tc.tile_pool(name="kpool", bufs=4))
        rqpool = ctx.enter_context(tc.tile_pool(name="rqpool", bufs=4))
        apool = ctx.enter_context(tc.tile_pool(name="apool", bufs=3))
        npool = ctx.enter_context(tc.tile_pool(name="npool", bufs=3))
        ypool = ctx.enter_context(tc.tile_pool(name="ypool", bufs=3))
        dpool = ctx.enter_context(tc.tile_pool(name="dpool", bufs=2))
        rpool = ctx.enter_context(tc.tile_pool(name="rpool", bufs=2))
        gpool = ctx.enter_context(tc.tile_pool(name="gpool", bufs=2))
        ps_s1 = ctx.enter_context(tc.tile_pool(name="ps_s1", bufs=2, space="PSUM"))
        ps_vph = ctx.enter_context(tc.tile_pool(name="ps_vph", bufs=3, space="PSUM"))
        ps_a2 = ctx.enter_context(tc.tile_pool(name="ps_a2", bufs=2, space="PSUM"))
        ps_wo = ctx.enter_context(tc.tile_pool(name="ps_wo", bufs=1, space="PSUM"))

        def wtile(dram, shape, dt):
            t = consts.tile(shape, dt, tag=f"w_{dram.name}")
            # Act HWDGE ring: overlaps with the big input DMAs on the SP ring
            nc.scalar.dma_start(out=t, in_=dram[:])
            return t

        # s1-gating weights first: they bound the first matmul's start
        Wka = wtile(w_ka, [D, H], BF16)
        P2aI = wtile(w_p2ai, [2 * H, H], BF16)
        Wkv = wtile(w_kv, [D, D], BF16)
        P2 = wtile(w_p2, [H, D], BF16)
        A2m = wtile(w_a2m, [D, D], BF16)
        Wo = wtile(w_o, [D, D], F32R)
        Bo = wtile(b_o, [D, 1], F32)

        kts = {}
        rqs = {}

        def phase1(c):
            """DMA + s1 matmuls for chunk c into the stack PSUM."""
            bi = c // DMAB
            if bi not in kts:
                kt = kpool.tile([D, DMAB * CHUNK], BF16, tag="kt")
                nc.sync.dma_start(
                    out=kt, in_=kf[:, bi * DMAB * CHUNK:(bi + 1) * DMAB * CHUNK])
                rqt = rqpool.tile([2 * H, DMAB * CHUNK], BF16, tag="rq")
                nc.sync.dma_start(
                    out=rqt, in_=rqf[:, bi * DMAB * CHUNK:(bi + 1) * DMAB * CHUNK])
                kts[bi] = kt
                rqs[bi] = rqt
            return kts[bi], rqs[bi]

        group_tiles = {}

        def emit_tail(g, xsup, den_g, ynum_g):
            rec = rpool.tile([D, TOK_GRP], F32, tag="rec")
            nc.vector.reciprocal(out=rec[:], in_=den_g[:])
            nc.vector.tensor_tensor(out=xsup[:], in0=ynum_g[:],
                                    in1=rec[:], op=ALU.mult)
            wo_ps = ps_wo.tile([D, TOK_GRP], F32, tag="wo")
            nc.tensor.matmul(wo_ps[:], Wo[:], xsup[:], start=True, stop=True)
            outt = gpool.tile([D, TOK_GRP], F32, tag="outt")
            nc.scalar.activation(outt[:], wo_ps[:], AF.Identity, bias=Bo[:])
            nc.scalar.dma_start(out=out_f[:, g * TOK_GRP:(g + 1) * TOK_GRP],
                              in_=outt[:])

        # first two stacks are small (1+3 chunks) so the first relu -> a2 ->
        # exp chain starts after 2 cold matmuls instead of 8
        stack_ranges = [(0, 1), (1, 4)] + [(s, s + STACK)
                                           for s in range(4, NCHUNK, STACK)]
        for rep in range(reps):
          kts.clear()
          rqs.clear()
          for (lo, hi) in stack_ranges:
            cs = list(range(lo, hi))

            # ---- phase 1: s1 matmuls into the stacked PSUM tile ----
            s1_ps = ps_s1.tile([D, CHUNK], F32, tag="s1stack")
            for ci, c in enumerate(cs):
                kt, rqt = phase1(c)
                off = (c % DMAB) * CHUNK
                o = ci * SLOT
                nc.tensor.matmul(s1_ps[o:o + H, :], Wka[:],
                                 kt[:, off:off + CHUNK], start=True, stop=False,
                                 tile_position=(0, o))
                nc.tensor.matmul(s1_ps[o:o + H, :], P2aI[:],
                                 rqt[:, off:off + CHUNK], start=False, stop=True,
                                 tile_position=(0, o))

            # ---- stacked relu + mask rows ----
            a1 = apool.tile([D, CHUNK], BF16, tag="a1stack")
            nc.scalar.activation(a1[:], s1_ps[:], AF.Relu)
            # mask rows at partitions o+H (o = 0/32/64/96)
            mrow_dst = a1[:].rearrange("(a b) f -> a b f", b=SLOT)[:, H:H + 1, :] \
                .rearrange("a b f -> (a b) f")
            nc.scalar.dma_start(out=mrow_dst, in_=mrows[cs[0]:cs[0] + STACK, :])

            # ---- phase 2 per chunk ----
            for ci, c in enumerate(cs):
                kt, rqt = kts[c // DMAB], rqs[c // DMAB]
                off = (c % DMAB) * CHUNK
                g = c // GRP
                t0 = (c % GRP) * TOK_CHUNK

                if c % GRP == 0:
                    xsup = gpool.tile([D, TOK_GRP], F32R, tag="xsup")

                vph_ps = ps_vph.tile([D, CHUNK], F32, tag="vph")
                nc.tensor.matmul(vph_ps[:], Wkv[:], kt[:, off:off + CHUNK],
                                 start=True, stop=False)
                nc.tensor.matmul(vph_ps[:], P2[:],
                                 rqt[0:H, off:off + CHUNK], start=False, stop=True)

                a2_ps = ps_a2.tile([D, CHUNK], F32, tag="a2")
                o = ci * SLOT
                nc.tensor.matmul(a2_ps[:], A2m[o:o + H + 1, :],
                                 a1[o:o + H + 1, :], start=True, stop=True,
                                 tile_position=(o, 0))

                if c % 2 == 0:
                    num2 = npool.tile([D, 2 * CHUNK], BF16, tag="num")
                    y2 = ypool.tile([D, 2 * CHUNK], BF16, tag="y")
                    pair = (num2, y2)
                num2, y2 = pair
                half = (c % 2) * CHUNK
                nc.scalar.activation(num2[:, half:half + CHUNK], a2_ps[:],
                                     AF.Exp)
                if c in uadd_chunks:
                    uat = npool.tile([1, CHUNK], BF16, tag="uadd")
                    nc.sync.dma_start(out=uat, in_=uadd[:, c * CHUNK:(c + 1) * CHUNK])
                    ub = uat[:].partition_broadcast(D).rearrange("p q f -> p (q f)")
                    nc.vector.tensor_tensor(out=num2[:, half:half + CHUNK],
                                            in0=num2[:, half:half + CHUNK],
                                            in1=ub, op=ALU.add)

                if c % GRP == 0:
                    den_g = gpool.tile([D, TOK_GRP], F32, tag="den_g")
                    ynum_g = gpool.tile([D, TOK_GRP], F32, tag="ynum_g")

                nc.vector.tensor_tensor(out=y2[:, half:half + CHUNK],
                                        in0=vph_ps[:],
                                        in1=num2[:, half:half + CHUNK],
                                        op=ALU.mult)

                if c % 2 == 1:
                    tp = t0 - TOK_CHUNK   # pair covers tokens [tp, tp+64)
                    # den: 4-level binary tree on GPSIMD over BOTH chunks
                    num3 = num2[:].rearrange("p (a b) -> p a b", b=K)
                    # last pairs run on DVE (idle at drain) so Pool's
                    # end-of-kernel backlog doesn't set the finish time
                    dtree = nc.vector if c >= NCHUNK - 4 else nc.gpsimd
                    t1 = dpool.tile([D, CHUNK], BF16, tag="dt1")
                    t13 = t1[:].rearrange("p (a b) -> p a b", b=K // 2)
                    dtree.tensor_tensor(out=t13, in0=num3[:, :, 0:8],
                                        in1=num3[:, :, 8:16], op=ALU.add)
                    t2 = dpool.tile([D, CHUNK // 2], BF16, tag="dt2")
                    t23 = t2[:].rearrange("p (a b) -> p a b", b=K // 4)
                    dtree.tensor_tensor(out=t23, in0=t13[:, :, 0:4],
                                        in1=t13[:, :, 4:8], op=ALU.add)
                    t3 = dpool.tile([D, CHUNK // 4], BF16, tag="dt3")
                    t33 = t3[:].rearrange("p (a b) -> p a b", b=K // 8)
                    dtree.tensor_tensor(out=t33, in0=t23[:, :, 0:2],
                                        in1=t23[:, :, 2:4], op=ALU.add)
                    dg3 = den_g[:, tp:tp + 2 * TOK_CHUNK] \
                        .rearrange("p (a b) -> p a b", b=1)
                    dtree.tensor_tensor(out=dg3, in0=t33[:, :, 0:1],
                                        in1=t33[:, :, 1:2], op=ALU.add)

                    # ynum: bf16 TT-add tree on DVE (2x mode beats the
                    # modeless tensor_reduce: 919 vs 1161 ns per pair)
                    y3 = y2[:].rearrange("p (a b) -> p a b", b=K)
                    w1 = dpool.tile([D, CHUNK], BF16, tag="wt1")
                    w13 = w1[:].rearrange("p (a b) -> p a b", b=K // 2)
                    nc.vector.tensor_tensor(out=w13, in0=y3[:, :, 0:8],
                                            in1=y3[:, :, 8:16], op=ALU.add)
                    w2 = dpool.tile([D, CHUNK // 2], BF16, tag="wt2")
                    w23 = w2[:].rearrange("p (a b) -> p a b", b=K // 4)
                    nc.vector.tensor_tensor(out=w23, in0=w13[:, :, 0:4],
                                            in1=w13[:, :, 4:8], op=ALU.add)
                    w3 = dpool.tile([D, CHUNK // 4], BF16, tag="wt3")
                    w33 = w3[:].rearrange("p (a b) -> p a b", b=K // 8)
                    nc.vector.tensor_tensor(out=w33, in0=w23[:, :, 0:2],
                                            in1=w23[:, :, 2:4], op=ALU.add)
                    ug3 = ynum_g[:, tp:tp + 2 * TOK_CHUNK] \
                        .rearrange("p (a b) -> p a b", b=1)
                    nc.vector.tensor_tensor(out=ug3, in0=w33[:, :, 0:1],
                                            in1=w33[:, :, 1:2], op=ALU.add)

                if c % GRP == GRP - 1:
                    group_tiles[g] = (xsup, den_g, ynum_g)
                # deferred group tail: emit 3 chunks into the next group so
                # the in-order PE queue never waits on the Pool/DVE tail chain
                gprev = g - 1
                if c % GRP == 3 and gprev in group_tiles:
                    emit_tail(gprev, *group_tiles.pop(gprev))

          while group_tiles:
            g = min(group_tiles)
            emit_tail(g, *group_tiles.pop(g))

    _legalize_waits(nc)
    return nc


_CACHE = {}


def kernel(q, k, pos, mask, Wq, Wk, Wv, P1, pb1, P2, pb2,
           A1, ab1, A2, ab2, Wo, bo):
    import ml_dtypes
    bf16 = ml_dtypes.bfloat16

    q = np.asarray(q, np.float32)
    k = np.asarray(k, np.float32)
    pos = np.asarray(pos, np.float32)
    mask_np = np.asarray(mask)
    Wq, Wk, Wv = (np.asarray(x, np.float32) for x in (Wq, Wk, Wv))
    P1, pb1, P2, pb2 = (np.asarray(x, np.float32) for x in (P1, pb1, P2, pb2))
    A1, ab1, A2, ab2 = (np.asarray(x, np.float32) for x in (A1, ab1, A2, ab2))
    Wo, bo = np.asarray(Wo, np.float32), np.asarray(bo, np.float32)

    # ---- host-side input prep (layout + weight/bias folding) ----
    R = T_TOTAL * K
    kT = np.ascontiguousarray(k.reshape(R, D).T)                      # [D, R]
    r1 = np.maximum(pos.reshape(R, 4) @ P1 + pb1, 0.0)                # [R, H]
    qab = q.reshape(T_TOTAL, D) @ (-(Wq @ A1)) + (ab1 + pb2 @ A1)     # [T, H]
    qab_r = np.repeat(qab, K, axis=0)                                 # [R, H]
    rqf = np.ascontiguousarray(
        np.concatenate([r1, qab_r], axis=1).T.astype(bf16))           # [2H, R]

    m = mask_np.reshape(T_TOTAL, K) != 0
    mc = (~m).astype(np.float32).reshape(-1)                          # [R]
    all_masked = ~m.any(axis=1)

    w_kv = np.ascontiguousarray((Wk @ Wv).astype(bf16))
    w_ka = np.ascontiguousarray((Wk @ A1).astype(bf16))
    w_p2ai = np.ascontiguousarray(
        np.concatenate([P2 @ A1, np.eye(H, dtype=np.float32)], axis=0)
        .astype(bf16))                                                # [2H, H]
    w_p2 = np.ascontiguousarray(P2.astype(bf16))
    a2m_blk = np.concatenate([A2, np.full((1, D), -1e9, np.float32)], axis=0)
    w_a2m = np.zeros((D, D), np.float32)                              # [D, D]
    for o in range(0, D, SLOT):
        w_a2m[o:o + H + 1] = a2m_blk
    w_a2m = np.ascontiguousarray(w_a2m.astype(bf16))
    b_o = (pb2 @ Wo + bo).reshape(D, 1)

    # all-masked tokens leak a uniform weight (matches softmax of all -1e9)
    uaddv = np.repeat(all_masked.astype(np.float32), K).reshape(1, -1).astype(bf16)
    uadd_chunks = set()
    if all_masked.any():
        for t in np.nonzero(all_masked)[0]:
            core = t // T_CORE
            local_tok = t - core * T_CORE
            uadd_chunks.add(local_tok // TOK_CHUNK)

    key = ("final2", tuple(sorted(uadd_chunks)))
    if key not in _CACHE:
        _CACHE[key] = _build_program(uadd_chunks)
    nc = _CACHE[key]

    shared = {
        "w_kv": w_kv, "w_ka": w_ka, "w_p2ai": w_p2ai,
        "w_p2": w_p2, "w_a2m": w_a2m,
        "w_o": np.ascontiguousarray(Wo), "b_o": b_o,
    }
    in_maps = []
    for c in range(NCORES):
        rs, re = c * R_CORE, (c + 1) * R_CORE
        im = dict(shared)
        im["kf"] = np.ascontiguousarray(kT[:, rs:re].astype(bf16))
        im["rqf"] = np.ascontiguousarray(rqf[:, rs:re])
        im["mrows"] = np.ascontiguousarray(
            mc[rs:re].reshape(NCHUNK, CHUNK).astype(bf16))
        im["uadd"] = np.ascontiguousarray(uaddv[:, rs:re])
        in_maps.append(im)

    res = run_bass_kernel_spmd(nc, in_maps, core_ids=list(range(NCORES)))
    kernel._last_results = res
    out = np.concatenate([res.results[c]["out_f"] for c in range(NCORES)],
                         axis=1)                        # [D, T]
    return np.ascontiguousarray(out.T).reshape(B, N, D).astype(np.float32)
